# revision 9
# baseline (speedup 1.0000x reference)
"""MeanTopKPooling2D (scrambled top-4-of-9 mean) Trainium2 kernel.

Math: the reference's "faithful reshape" makes output position s = i2*110+j2
take candidates patches_flat[q*12100 + s] for q = 0..8 (patches_flat in
window-major (i, j, t) order, per channel).  With U[L, j2] :=
patches_flat[110*L + j2] (shape [990, 110] per channel), candidate stream q
is rows [110q, 110q+110) of U, and:

    U[9u + e, phi + 9k] = x[u + t//3, j0(e,t) + k]      for t, e in [0,9)
    phi = (t - 2e) mod 9,  j0 = (110e + phi - t)//9 + t%3

(verified numerically against the reference).  So U is built with 81 regular
strided DMAs per batch and the candidate tiles are then contiguous row-block
slices of U.  Selection is an exact min/max network: sort4 + sort4, prefix
sums, and sum_top4(A u B u {y}) = max(max_j PA_j+PB_{4-j},
y + max_j PA_j+PB_{3-j}).

Layout: partition = output row i2 (110), free = (j2-chunk, c=128).
Data-parallel over batch: core k handles batches [2k, 2k+2).
"""

import numpy as np
from contextlib import ExitStack

_NC_CACHE = {}

B_PER_CORE = 2
H = W = 112
NH = NW = 110
C = 128
JC = 27  # j2 chunk width (multiple of 9 keeps stage-1 runs unfragmented)
CHUNKS = [(0, 27), (27, 27), (54, 27), (81, 27), (108, 2)]


class _Slots:
    """Rotating slot allocator over one tile pool (bounds SBUF footprint)."""

    def __init__(self, tc, pool, shape, dtype, n):
        self.pool = pool
        self.shape = shape
        self.dtype = dtype
        self.free = [f"w{i}" for i in range(n)]

    def alloc(self):
        tag = self.free.pop()
        t = self.pool.tile(self.shape, self.dtype, tag=tag)
        return (tag, t)

    def release(self, *handles):
        for tag, _ in handles:
            self.free.append(tag)


def _build_nc():
    import concourse.bass as bass
    import concourse.bacc as bacc
    import concourse.tile as tile
    from concourse import mybir

    f32 = mybir.dt.float32
    MAX, MIN, ADD = (
        mybir.AluOpType.max,
        mybir.AluOpType.min,
        mybir.AluOpType.add,
    )

    # TRN2 instructions carry at most ONE sync-wait command; Bacc's
    # compile() pass (generate_event_semaphores) splits multi-waits into
    # event-semaphore chains, which plain Bass does not do.
    nc = bacc.Bacc("TRN2", target_bir_lowering=False, debug=False)
    x = nc.declare_dram_parameter("x", [B_PER_CORE, H, W, C], f32, isOutput=False)
    out = nc.declare_dram_parameter(
        "out", [B_PER_CORE, NH, NW, C], f32, isOutput=True
    )

    with tile.TileContext(nc) as tc, ExitStack() as ctx:
        upool = ctx.enter_context(tc.tile_pool(name="u", bufs=2, space="DRAM"))
        wpool = ctx.enter_context(tc.tile_pool(name="w", bufs=1))
        opool = ctx.enter_context(tc.tile_pool(name="o", bufs=2))

        for b in range(B_PER_CORE):
            for (j0c, jc) in CHUNKS:
                # ---- stage 1: build U chunk [990, jc, 128] in DRAM ----
                U = upool.tile([9 * NH, jc, C], f32, tag="U")
                for e in range(9):
                    for t in range(9):
                        phi = (t - 2 * e) % 9
                        dr, dc = t // 3, t % 3
                        j0 = (110 * e + phi - t) // 9 + dc
                        # global k range covering j2 in [j0c, j0c+jc)
                        k0 = (j0c - phi + 8) // 9
                        k1 = (j0c + jc - phi + 8) // 9  # exclusive
                        if k1 <= k0:
                            continue
                        src = x.ap()[b, dr : dr + NH, j0 + k0 : j0 + k1, :]
                        phl = phi + 9 * k0 - j0c
                        dst = U[e::9, phl::9, :]
                        nc.sync.dma_start(out=dst, in_=src)

                # ---- stage 2 + network ----
                sl = _Slots(tc, wpool, [NH, jc, C], f32, 13)

                V = []
                for q in range(9):
                    h = sl.alloc()
                    nc.sync.dma_start(
                        out=h[1][:, :, :], in_=U[110 * q : 110 * (q + 1), :, :]
                    )
                    V.append(h)

                def op3(op, p, q):
                    r = sl.alloc()
                    nc.vector.tensor_tensor(r[1][:, :, :], p[1][:, :, :], q[1][:, :, :], op)
                    return r

                def cx(p, q):
                    h, l = op3(MAX, p, q), op3(MIN, p, q)
                    sl.release(p, q)
                    return h, l

                def sort4(t0, t1, t2, t3):
                    s1, s2 = cx(t0, t1)
                    s3, s4 = cx(t2, t3)
                    a1, h2 = cx(s1, s3)
                    l1, a4 = cx(s2, s4)
                    a2, a3 = cx(h2, l1)
                    return a1, a2, a3, a4

                def add_consuming(p, q):
                    r = op3(ADD, p, q)
                    sl.release(p, q)
                    return r

                a1, a2, a3, a4 = sort4(V[0], V[1], V[2], V[3])
                b1, b2, b3, b4 = sort4(V[4], V[5], V[6], V[7])

                PA2 = op3(ADD, a1, a2); sl.release(a2)
                PA3 = op3(ADD, PA2, a3); sl.release(a3)
                PA4 = op3(ADD, PA3, a4); sl.release(a4)
                PB2 = op3(ADD, b1, b2); sl.release(b2)
                PB3 = op3(ADD, PB2, b3); sl.release(b3)
                PB4 = op3(ADD, PB3, b4); sl.release(b4)

                m1 = op3(MAX, PA4, PB4); sl.release(PA4, PB4)
                u1 = op3(ADD, a1, PB3)
                u2 = op3(ADD, PA2, PB2)
                u3 = op3(ADD, PA3, b1)
                m2 = op3(MAX, u1, u2); sl.release(u1, u2)
                m3 = op3(MAX, m2, u3); sl.release(m2, u3)
                S4 = op3(MAX, m1, m3); sl.release(m1, m3)

                w1 = op3(ADD, a1, PB2); sl.release(a1, PB2)
                w2 = op3(ADD, PA2, b1); sl.release(PA2, b1)
                x1 = op3(MAX, PA3, PB3); sl.release(PA3, PB3)
                x2 = op3(MAX, w1, w2); sl.release(w1, w2)
                S3 = op3(MAX, x1, x2); sl.release(x1, x2)

                Z = add_consuming(V[8], S3)
                R = op3(MAX, S4, Z); sl.release(S4, Z)

                o = opool.tile([NH, jc, C], f32, tag="out")
                nc.vector.tensor_scalar_mul(o[:, :, :], R[1][:, :, :], 0.25)
                sl.release(R)
                nc.sync.dma_start(
                    out=out.ap()[b, :, j0c : j0c + jc, :], in_=o[:, :, :]
                )
    nc.compile()
    return nc


def kernel(x: np.ndarray) -> np.ndarray:
    from concourse.bass_utils import run_bass_kernel_spmd

    if "nc" not in _NC_CACHE:
        _NC_CACHE["nc"] = _build_nc()
    nc = _NC_CACHE["nc"]

    n_cores = 8
    xs = np.ascontiguousarray(x, dtype=np.float32)
    in_maps = [
        {"x": np.ascontiguousarray(xs[k * B_PER_CORE : (k + 1) * B_PER_CORE])}
        for k in range(n_cores)
    ]
    res = run_bass_kernel_spmd(nc, in_maps, core_ids=list(range(n_cores)))
    return np.concatenate([res.results[k]["out"] for k in range(n_cores)], axis=0)


# revision 10
# speedup vs baseline: 1.1186x; 1.1186x over previous
"""MeanTopKPooling2D (scrambled top-4-of-9 mean) Trainium2 kernel.

Math: the reference's "faithful reshape" makes output position s = i2*110+j2
take candidates patches_flat[q*12100 + s] for q = 0..8 (patches_flat in
window-major (i, j, t) order, per channel).  With U[L, j2] :=
patches_flat[110*L + j2] (shape [990, 110] per channel), candidate stream q
is rows [110q, 110q+110) of U, and:

    U[9u + e, phi + 9k] = x[u + t//3, j0(e,t) + k]      for t, e in [0,9)
    phi = (t - 2e) mod 9,  j0 = (110e + phi - t)//9 + t%3

(verified numerically against the reference).  So U is built with 81 regular
strided DMAs per batch and the candidate tiles are then contiguous row-block
slices of U.  Selection is an exact min/max network: sort4 + sort4, prefix
sums, and sum_top4(A u B u {y}) = max(max_j PA_j+PB_{4-j},
y + max_j PA_j+PB_{3-j}).

Layout: partition = output row i2 (110), free = (j2-chunk, c=128).
Data-parallel over batch: core k handles batches [2k, 2k+2).
"""

import numpy as np
from contextlib import ExitStack

_NC_CACHE = {}

B_PER_CORE = 2
H = W = 112
NH = NW = 110
C = 128
JC = 27  # j2 chunk width (multiple of 9 keeps stage-1 runs unfragmented)
CHUNKS = [(0, 27), (27, 27), (54, 27), (81, 27), (108, 2)]


class _Slots:
    """Rotating slot allocator over one tile pool (bounds SBUF footprint)."""

    def __init__(self, tc, pool, shape, dtype, n):
        self.pool = pool
        self.shape = shape
        self.dtype = dtype
        self.free = [f"w{i}" for i in range(n)]

    def alloc(self):
        tag = self.free.pop()
        t = self.pool.tile(self.shape, self.dtype, tag=tag)
        return (tag, t)

    def release(self, *handles):
        for tag, _ in handles:
            self.free.append(tag)


def _build_nc():
    import concourse.bass as bass
    import concourse.bacc as bacc
    import concourse.tile as tile
    from concourse import mybir

    f32 = mybir.dt.float32
    MAX, MIN, ADD = (
        mybir.AluOpType.max,
        mybir.AluOpType.min,
        mybir.AluOpType.add,
    )

    # TRN2 instructions carry at most ONE sync-wait command; Bacc's
    # compile() pass (generate_event_semaphores) splits multi-waits into
    # event-semaphore chains, which plain Bass does not do.
    nc = bacc.Bacc("TRN2", target_bir_lowering=False, debug=False)
    x = nc.declare_dram_parameter("x", [B_PER_CORE, H, W, C], f32, isOutput=False)
    out = nc.declare_dram_parameter(
        "out", [B_PER_CORE, NH, NW, C], f32, isOutput=True
    )

    with tile.TileContext(nc) as tc, ExitStack() as ctx:
        upool = ctx.enter_context(tc.tile_pool(name="u", bufs=2, space="DRAM"))
        wpool = ctx.enter_context(tc.tile_pool(name="w", bufs=1))
        opool = ctx.enter_context(tc.tile_pool(name="o", bufs=2))

        for b in range(B_PER_CORE):
            for (j0c, jc) in CHUNKS:
                # ---- stage 1: build U chunk [990, jc, 128] in DRAM ----
                U = upool.tile([9 * NH, jc, C], f32, tag="U")
                for e in range(9):
                    for t in range(9):
                        phi = (t - 2 * e) % 9
                        dr, dc = t // 3, t % 3
                        j0 = (110 * e + phi - t) // 9 + dc
                        # global k range covering j2 in [j0c, j0c+jc)
                        k0 = (j0c - phi + 8) // 9
                        k1 = (j0c + jc - phi + 8) // 9  # exclusive
                        if k1 <= k0:
                            continue
                        src = x.ap()[b, dr : dr + NH, j0 + k0 : j0 + k1, :]
                        phl = phi + 9 * k0 - j0c
                        dst = U[e::9, phl::9, :]
                        nc.sync.dma_start(out=dst, in_=src)

                # ---- stage 2 + network ----
                sl = _Slots(tc, wpool, [NH, jc, C], f32, 13)

                V = []
                for q in range(9):
                    h = sl.alloc()
                    nc.sync.dma_start(
                        out=h[1][:, :, :], in_=U[110 * q : 110 * (q + 1), :, :]
                    )
                    V.append(h)

                def op3(op, p, q):
                    r = sl.alloc()
                    nc.vector.tensor_tensor(r[1][:, :, :], p[1][:, :, :], q[1][:, :, :], op)
                    return r

                def cx(p, q):
                    h, l = op3(MAX, p, q), op3(MIN, p, q)
                    sl.release(p, q)
                    return h, l

                def sort4(t0, t1, t2, t3):
                    s1, s2 = cx(t0, t1)
                    s3, s4 = cx(t2, t3)
                    a1, h2 = cx(s1, s3)
                    l1, a4 = cx(s2, s4)
                    a2, a3 = cx(h2, l1)
                    return a1, a2, a3, a4

                def add_consuming(p, q):
                    r = op3(ADD, p, q)
                    sl.release(p, q)
                    return r

                a1, a2, a3, a4 = sort4(V[0], V[1], V[2], V[3])
                b1, b2, b3, b4 = sort4(V[4], V[5], V[6], V[7])

                PA2 = op3(ADD, a1, a2); sl.release(a2)
                PA3 = op3(ADD, PA2, a3); sl.release(a3)
                PA4 = op3(ADD, PA3, a4); sl.release(a4)
                PB2 = op3(ADD, b1, b2); sl.release(b2)
                PB3 = op3(ADD, PB2, b3); sl.release(b3)
                PB4 = op3(ADD, PB3, b4); sl.release(b4)

                m1 = op3(MAX, PA4, PB4); sl.release(PA4, PB4)
                u1 = op3(ADD, a1, PB3)
                u2 = op3(ADD, PA2, PB2)
                u3 = op3(ADD, PA3, b1)
                m2 = op3(MAX, u1, u2); sl.release(u1, u2)
                m3 = op3(MAX, m2, u3); sl.release(m2, u3)
                S4 = op3(MAX, m1, m3); sl.release(m1, m3)

                w1 = op3(ADD, a1, PB2); sl.release(a1, PB2)
                w2 = op3(ADD, PA2, b1); sl.release(PA2, b1)
                x1 = op3(MAX, PA3, PB3); sl.release(PA3, PB3)
                x2 = op3(MAX, w1, w2); sl.release(w1, w2)
                S3 = op3(MAX, x1, x2); sl.release(x1, x2)

                Z = add_consuming(V[8], S3)
                R = op3(MAX, S4, Z); sl.release(S4, Z)

                o = opool.tile([NH, jc, C], f32, tag="out")
                nc.vector.tensor_scalar_mul(o[:, :, :], R[1][:, :, :], 0.25)
                sl.release(R)
                nc.sync.dma_start(
                    out=out.ap()[b, :, j0c : j0c + jc, :], in_=o[:, :, :]
                )
    nc.compile()
    return nc


def _get_runner():
    """Compile once; reuse the jitted shard_map executable across calls.

    run_bass_kernel_spmd rebuilds its jit closure per call (full retrace +
    XLA compile each time); this replicates its multi-core branch with a
    persistent compiled function.
    """
    if "runner" in _NC_CACHE:
        return _NC_CACHE["runner"]

    import jax
    from jax.sharding import Mesh, PartitionSpec
    from jax.experimental.shard_map import shard_map
    from concourse import mybir
    from concourse.bass2jax import (
        _bass_exec_p,
        install_neuronx_cc_hook,
        partition_id_tensor,
    )

    nc = _NC_CACHE.get("nc")
    if nc is None:
        nc = _NC_CACHE["nc"] = _build_nc()
    install_neuronx_cc_hook()

    n_cores = 8
    partition_name = nc.partition_id_tensor.name if nc.partition_id_tensor else None
    in_names, out_names, out_avals, zero_shapes = [], [], [], []
    for alloc in nc.m.functions[0].allocations:
        if not isinstance(alloc, mybir.MemoryLocationSet):
            continue
        name = alloc.memorylocations[0].name
        if alloc.kind == "ExternalInput":
            if name != partition_name:
                in_names.append(name)
        elif alloc.kind == "ExternalOutput":
            out_names.append(name)
            shape = tuple(alloc.tensor_shape)
            dtype = mybir.dt.np(alloc.dtype)
            out_avals.append(jax.core.ShapedArray(shape, dtype))
            zero_shapes.append((shape, dtype))
    n_params = len(in_names)
    n_outs = len(out_avals)
    all_in_names = list(in_names) + list(out_names)
    if partition_name is not None:
        all_in_names.append(partition_name)
    donate = tuple(range(n_params, n_params + n_outs))

    def _body(*args):
        operands = list(args)
        if partition_name is not None:
            operands.append(partition_id_tensor())
        return tuple(
            _bass_exec_p.bind(
                *operands,
                out_avals=tuple(out_avals),
                in_names=tuple(all_in_names),
                out_names=tuple(out_names),
                lowering_input_output_aliases=(),
                sim_require_finite=True,
                sim_require_nnan=True,
                nc=nc,
            )
        )

    devices = jax.devices()[:n_cores]
    mesh = Mesh(np.asarray(devices), ("core",))
    in_specs = (PartitionSpec("core"),) * (n_params + n_outs)
    out_specs = (PartitionSpec("core"),) * n_outs
    sharded = jax.jit(
        shard_map(
            _body, mesh=mesh, in_specs=in_specs, out_specs=out_specs, check_rep=False
        ),
        donate_argnums=donate,
        keep_unused=True,
    )

    def run(in_maps):
        concat_in = [
            np.concatenate([np.asarray(m[name]) for m in in_maps], axis=0)
            for name in in_names
        ]
        concat_zeros = [
            np.zeros((n_cores * s[0], *s[1:]), d) for (s, d) in zero_shapes
        ]
        out_arrs = sharded(*concat_in, *concat_zeros)
        return [
            {
                name: np.asarray(out_arrs[i]).reshape(
                    n_cores, *out_avals[i].shape
                )[c]
                for i, name in enumerate(out_names)
            }
            for c in range(n_cores)
        ]

    _NC_CACHE["runner"] = run
    return run


def kernel(x: np.ndarray) -> np.ndarray:
    run = _get_runner()
    n_cores = 8
    xs = np.ascontiguousarray(x, dtype=np.float32)
    in_maps = [
        {"x": xs[k * B_PER_CORE : (k + 1) * B_PER_CORE]} for k in range(n_cores)
    ]
    results = run(in_maps)
    return np.concatenate([results[k]["out"] for k in range(n_cores)], axis=0)


# revision 14
# speedup vs baseline: 1.1333x; 1.0131x over previous
"""MeanTopKPooling2D (scrambled top-4-of-9 mean) Trainium2 kernel.

Math: the reference's "faithful reshape" makes output position s = i2*110+j2
take candidates patches_flat[q*12100 + s] for q = 0..8 (patches_flat in
window-major (i, j, t) order, per channel).  With U[L, j2] :=
patches_flat[110*L + j2] (shape [990, 110] per channel), candidate stream q
is rows [110q, 110q+110) of U, and:

    U[9u + e, phi + 9k] = x[u + t//3, j0(e,t) + k]      for t, e in [0,9)
    phi = (t - 2e) mod 9,  j0 = (110e + phi - t)//9 + t%3

(verified numerically against the reference).  So U is built with 81 regular
strided DMAs per batch and the candidate tiles are then contiguous row-block
slices of U.  Selection is an exact min/max network: sort4 + sort4, prefix
sums, and sum_top4(A u B u {y}) = max(max_j PA_j+PB_{4-j},
y + max_j PA_j+PB_{3-j}).

Layout: partition = output row i2 (110), free = (j2-chunk, c=128).
Data-parallel over batch: core k handles batches [2k, 2k+2).
"""

import numpy as np
from contextlib import ExitStack

_NC_CACHE = {}

B_PER_CORE = 2
H = W = 112
NH = NW = 110
C = 128
JC = 18  # j2 chunk width (multiple of 9 keeps stage-1 runs unfragmented)
CHUNKS = [(0, 18), (18, 18), (36, 18), (54, 18), (72, 18), (90, 18), (108, 2)]


class _Slots:
    """Rotating slot allocator over one tile pool (bounds SBUF footprint)."""

    def __init__(self, tc, pool, shape, dtype, n):
        self.pool = pool
        self.shape = shape
        self.dtype = dtype
        self.free = [f"w{i}" for i in range(n)]

    def alloc(self):
        tag = self.free.pop()
        t = self.pool.tile(self.shape, self.dtype, tag=tag)
        return (tag, t)

    def release(self, *handles):
        for tag, _ in handles:
            self.free.append(tag)


def _build_nc():
    import concourse.bass as bass
    import concourse.bacc as bacc
    import concourse.tile as tile
    from concourse import mybir

    f32 = mybir.dt.float32
    MAX, MIN, ADD = (
        mybir.AluOpType.max,
        mybir.AluOpType.min,
        mybir.AluOpType.add,
    )

    # TRN2 instructions carry at most ONE sync-wait command; Bacc's
    # compile() pass (generate_event_semaphores) splits multi-waits into
    # event-semaphore chains, which plain Bass does not do.
    nc = bacc.Bacc("TRN2", target_bir_lowering=False, debug=False)
    x = nc.declare_dram_parameter("x", [B_PER_CORE, H, W, C], f32, isOutput=False)
    out = nc.declare_dram_parameter(
        "out", [B_PER_CORE, NH, NW, C], f32, isOutput=True
    )

    with tile.TileContext(nc) as tc, ExitStack() as ctx:
        upool = ctx.enter_context(tc.tile_pool(name="u", bufs=2, space="DRAM"))
        wpool = ctx.enter_context(tc.tile_pool(name="w", bufs=1))
        opool = ctx.enter_context(tc.tile_pool(name="o", bufs=2))

        xpool = ctx.enter_context(tc.tile_pool(name="x", bufs=1))
        for b in range(B_PER_CORE):
            # Stage x[b] into SBUF once; the 81 stage-1 DMAs per chunk then
            # read SBUF instead of re-reading x from HBM ~9x (each pixel is
            # in up to 9 windows).
            xt = xpool.tile([H, W, C], f32, tag="xt")
            nc.sync.dma_start(out=xt[:, :, :], in_=x.ap()[b])

            for (j0c, jc) in CHUNKS:
                # ---- stage 1: build U chunk [990, jc, 128] in DRAM ----
                U = upool.tile([9 * NH, jc, C], f32, tag="U")
                for e in range(9):
                    for t in range(9):
                        phi = (t - 2 * e) % 9
                        dr, dc = t // 3, t % 3
                        j0 = (110 * e + phi - t) // 9 + dc
                        # global k range covering j2 in [j0c, j0c+jc)
                        k0 = (j0c - phi + 8) // 9
                        k1 = (j0c + jc - phi + 8) // 9  # exclusive
                        if k1 <= k0:
                            continue
                        src = xt[dr : dr + NH, j0 + k0 : j0 + k1, :]
                        phl = phi + 9 * k0 - j0c
                        dst = U[e::9, phl::9, :]
                        nc.sync.dma_start(out=dst, in_=src)

                # ---- stage 2 + network ----
                sl = _Slots(tc, wpool, [NH, jc, C], f32, 13)

                V = []
                for q in range(9):
                    h = sl.alloc()
                    nc.sync.dma_start(
                        out=h[1][:, :, :], in_=U[110 * q : 110 * (q + 1), :, :]
                    )
                    V.append(h)

                def op3(op, p, q):
                    r = sl.alloc()
                    nc.vector.tensor_tensor(r[1][:, :, :], p[1][:, :, :], q[1][:, :, :], op)
                    return r

                def cx(p, q):
                    h, l = op3(MAX, p, q), op3(MIN, p, q)
                    sl.release(p, q)
                    return h, l

                def sort4(t0, t1, t2, t3):
                    s1, s2 = cx(t0, t1)
                    s3, s4 = cx(t2, t3)
                    a1, h2 = cx(s1, s3)
                    l1, a4 = cx(s2, s4)
                    a2, a3 = cx(h2, l1)
                    return a1, a2, a3, a4

                def add_consuming(p, q):
                    r = op3(ADD, p, q)
                    sl.release(p, q)
                    return r

                a1, a2, a3, a4 = sort4(V[0], V[1], V[2], V[3])
                b1, b2, b3, b4 = sort4(V[4], V[5], V[6], V[7])

                PA2 = op3(ADD, a1, a2); sl.release(a2)
                PA3 = op3(ADD, PA2, a3); sl.release(a3)
                PA4 = op3(ADD, PA3, a4); sl.release(a4)
                PB2 = op3(ADD, b1, b2); sl.release(b2)
                PB3 = op3(ADD, PB2, b3); sl.release(b3)
                PB4 = op3(ADD, PB3, b4); sl.release(b4)

                m1 = op3(MAX, PA4, PB4); sl.release(PA4, PB4)
                u1 = op3(ADD, a1, PB3)
                u2 = op3(ADD, PA2, PB2)
                u3 = op3(ADD, PA3, b1)
                m2 = op3(MAX, u1, u2); sl.release(u1, u2)
                m3 = op3(MAX, m2, u3); sl.release(m2, u3)
                S4 = op3(MAX, m1, m3); sl.release(m1, m3)

                w1 = op3(ADD, a1, PB2); sl.release(a1, PB2)
                w2 = op3(ADD, PA2, b1); sl.release(PA2, b1)
                x1 = op3(MAX, PA3, PB3); sl.release(PA3, PB3)
                x2 = op3(MAX, w1, w2); sl.release(w1, w2)
                S3 = op3(MAX, x1, x2); sl.release(x1, x2)

                Z = add_consuming(V[8], S3)
                R = op3(MAX, S4, Z); sl.release(S4, Z)

                o = opool.tile([NH, jc, C], f32, tag="out")
                nc.vector.tensor_scalar_mul(o[:, :, :], R[1][:, :, :], 0.25)
                sl.release(R)
                nc.sync.dma_start(
                    out=out.ap()[b, :, j0c : j0c + jc, :], in_=o[:, :, :]
                )
    nc.compile()
    return nc


def _get_runner():
    """Compile once; reuse the jitted shard_map executable across calls.

    run_bass_kernel_spmd rebuilds its jit closure per call (full retrace +
    XLA compile each time); this replicates its multi-core branch with a
    persistent compiled function.
    """
    if "runner" in _NC_CACHE:
        return _NC_CACHE["runner"]

    import jax
    from jax.sharding import Mesh, PartitionSpec
    from jax.experimental.shard_map import shard_map
    from concourse import mybir
    from concourse.bass2jax import (
        _bass_exec_p,
        install_neuronx_cc_hook,
        partition_id_tensor,
    )

    nc = _NC_CACHE.get("nc")
    if nc is None:
        nc = _NC_CACHE["nc"] = _build_nc()
    install_neuronx_cc_hook()

    n_cores = 8
    partition_name = nc.partition_id_tensor.name if nc.partition_id_tensor else None
    in_names, out_names, out_avals, zero_shapes = [], [], [], []
    for alloc in nc.m.functions[0].allocations:
        if not isinstance(alloc, mybir.MemoryLocationSet):
            continue
        name = alloc.memorylocations[0].name
        if alloc.kind == "ExternalInput":
            if name != partition_name:
                in_names.append(name)
        elif alloc.kind == "ExternalOutput":
            out_names.append(name)
            shape = tuple(alloc.tensor_shape)
            dtype = mybir.dt.np(alloc.dtype)
            out_avals.append(jax.core.ShapedArray(shape, dtype))
            zero_shapes.append((shape, dtype))
    n_params = len(in_names)
    n_outs = len(out_avals)
    all_in_names = list(in_names) + list(out_names)
    if partition_name is not None:
        all_in_names.append(partition_name)
    donate = tuple(range(n_params, n_params + n_outs))

    def _body(*args):
        operands = list(args)
        if partition_name is not None:
            operands.append(partition_id_tensor())
        return tuple(
            _bass_exec_p.bind(
                *operands,
                out_avals=tuple(out_avals),
                in_names=tuple(all_in_names),
                out_names=tuple(out_names),
                lowering_input_output_aliases=(),
                sim_require_finite=True,
                sim_require_nnan=True,
                nc=nc,
            )
        )

    devices = jax.devices()[:n_cores]
    mesh = Mesh(np.asarray(devices), ("core",))
    in_specs = (PartitionSpec("core"),) * (n_params + n_outs)
    out_specs = (PartitionSpec("core"),) * n_outs
    sharded = jax.jit(
        shard_map(
            _body, mesh=mesh, in_specs=in_specs, out_specs=out_specs, check_rep=False
        ),
        donate_argnums=donate,
        keep_unused=True,
    )

    def run(in_maps):
        concat_in = [
            np.concatenate([np.asarray(m[name]) for m in in_maps], axis=0)
            for name in in_names
        ]
        concat_zeros = [
            np.zeros((n_cores * s[0], *s[1:]), d) for (s, d) in zero_shapes
        ]
        out_arrs = sharded(*concat_in, *concat_zeros)
        return [
            {
                name: np.asarray(out_arrs[i]).reshape(
                    n_cores, *out_avals[i].shape
                )[c]
                for i, name in enumerate(out_names)
            }
            for c in range(n_cores)
        ]

    _NC_CACHE["runner"] = run
    _NC_CACHE["sharded"] = sharded
    _NC_CACHE["meta"] = (in_names, out_names, out_avals, zero_shapes, mesh)
    return run


def kernel(x: np.ndarray) -> np.ndarray:
    run = _get_runner()
    n_cores = 8
    xs = np.ascontiguousarray(x, dtype=np.float32)
    in_maps = [
        {"x": xs[k * B_PER_CORE : (k + 1) * B_PER_CORE]} for k in range(n_cores)
    ]
    results = run(in_maps)
    return np.concatenate([results[k]["out"] for k in range(n_cores)], axis=0)
